# revision 4
# baseline (speedup 1.0000x reference)
"""BoxFilter 9x9 mean, TRN2 x8 — v6: prefix-scan horizontal, 2 wide matmuls.

Per 128-row input tile: one DVE tensor_tensor_scan builds the horizontal
inclusive cumsum C (fp32 state, f16 out) over the zero-padded row; the 9-tap
horizontal window is then h9[c] = C[c+9] - C[c], folded into the PE pass as
two 1024-wide matmuls with +W and -W banded weights (vertical 9-window and
1/(9*count_v) row normalization folded into the f16 weights). One Act
cast-copy PSUM->SBUF f16, one store DMA per block. Loads ride SWDGE (Pool),
stores ride SP HWDGE, so no engine queue is oversubscribed and the serialized
DMA-engine device (~360 B/ns, destination-side bytes) stays the bottleneck.
"""

import threading

import numpy as np

NCORES = 8
B, C, H, W = 16, 3, 1024, 1024
IMGS = B * C
IMGS_PER_CORE = IMGS // NCORES
R = 4
OB = 120  # output rows per full block
NFULL = H // OB  # 8 full blocks
LASTO = H - NFULL * OB  # 64
XPW = W + 16  # padded row width: 5 left zeros, x, 11 right (>=4 read) zeros
CW = W + 9  # cumsum width read by the matmuls

# per-image block table: (out_start, out_rows, in_start, in_rows, w_idx)
BLOCKS = []
BLOCKS.append((0, OB, 0, 124, 0))
for I in range(1, NFULL):
    BLOCKS.append((OB * I, OB, OB * I - R, 128, 1))
BLOCKS.append((H - LASTO, LASTO, H - 96, 96, 2))


def _window_counts():
    r = np.arange(H)
    return (np.minimum(r + R, H - 1) - np.maximum(r - R, 0) + 1).astype(np.float32)


def _consts():
    """Banded vertical-window weights with row normalization folded in.

    Layout [128, 768]: cols [wi*128, wi*128+orows) hold +W for block type wi,
    cols [384+wi*128, ...) hold -W. W[k, m] = 1/(9*count_v[row]) on the band.
    """
    ch = _window_counts()
    wts = np.zeros((128, 768), np.float16)
    for wi, (os_, orows, is_, irows) in (
        (0, (0, OB, 0, 124)),
        (1, (OB, OB, OB - R, 128)),
        (2, (H - LASTO, LASTO, H - 96, 96)),
    ):
        k = np.arange(irows)[:, None]
        m = np.arange(orows)[None, :]
        gr = os_ + m
        lo = np.maximum(gr - R, 0) - is_
        hi = np.minimum(gr + R, H - 1) - is_
        band = ((k >= lo) & (k <= hi)).astype(np.float32)
        w = (band * (1.0 / (9.0 * ch[os_ : os_ + orows]))[None, :]).astype(
            np.float16
        )
        wts[0:irows, wi * 128 : wi * 128 + orows] = w
        wts[0:irows, 384 + wi * 128 : 384 + wi * 128 + orows] = -w
    return wts


def _build(reps: int = 1):
    import concourse.bacc as bacc
    import concourse.mybir as mybir
    import concourse.tile as tile

    f32 = mybir.dt.float32
    f16 = mybir.dt.float16
    ADD = mybir.AluOpType.add

    nc = bacc.Bacc("TRN2", target_bir_lowering=False, debug=False, num_devices=NCORES)
    x_d = nc.declare_dram_parameter("x", [IMGS_PER_CORE, H, W], f32, isOutput=False)
    wts_d = nc.declare_dram_parameter("wts", [128, 768], f16, isOutput=False)
    o_d = nc.declare_dram_parameter("out", [IMGS_PER_CORE, H, W], f16, isOutput=True)

    NB = 8  # rotation depth for SBUF tiles
    NPS = 4  # PSUM tiles (2 banks each)

    with tile.TileContext(nc) as tc, (
        tc.tile_pool(name="consts", bufs=1)
    ) as cpool, tc.tile_pool(name="bufs", bufs=1) as bpool, (
        tc.tile_pool(name="psum", bufs=1, space="PSUM")
    ) as ppool:
        w_sb = cpool.tile([128, 768], f16, name="w_sb")
        nc.sync.dma_start(out=w_sb[:], in_=wts_d[:])
        zer = cpool.tile([128, 1036], f16, name="zer")
        nc.gpsimd.memset(zer[:], 0.0)

        xps = [bpool.tile([128, XPW], f16, name=f"xp{i}") for i in range(NB)]
        cts = [bpool.tile([128, 1036], f16, name=f"ct{i}") for i in range(NB)]
        obs = [bpool.tile([128, W], f16, name=f"ob{i}") for i in range(NB)]
        pss = [ppool.tile([128, W], f32, name=f"psm{i}") for i in range(NPS)]

        # pad columns are written once; loads only touch cols [5, W+5)
        for i in range(NB):
            nc.gpsimd.memset(xps[i][0:128, 0:5], 0.0)
            nc.gpsimd.memset(xps[i][0:128, W + 5 : XPW], 0.0)

        idx = 0
        for _ in range(reps):
            for g in range(IMGS_PER_CORE):
                for os_, orows, is_, irows, wi in BLOCKS:
                    xp = xps[idx % NB]
                    ct = cts[idx % NB]
                    ob = obs[idx % NB]
                    ps = pss[idx % NPS]
                    nc.gpsimd.dma_start(
                        out=xp[0:irows, 5 : W + 5], in_=x_d[g, is_ : is_ + irows, :]
                    )
                    nc.vector.tensor_tensor_scan(
                        ct[0:irows, 0:CW],
                        xp[0:irows, 0:CW],
                        zer[0:irows, 0:CW],
                        0.0,
                        ADD,
                        ADD,
                    )
                    for j0 in (0, 512):
                        nc.tensor.matmul(
                            ps[0:orows, j0 : j0 + 512],
                            w_sb[0:irows, wi * 128 : wi * 128 + orows],
                            ct[0:irows, j0 + 9 : j0 + 521],
                            start=True,
                            stop=False,
                        )
                        nc.tensor.matmul(
                            ps[0:orows, j0 : j0 + 512],
                            w_sb[0:irows, 384 + wi * 128 : 384 + wi * 128 + orows],
                            ct[0:irows, j0 : j0 + 512],
                            start=False,
                            stop=True,
                        )
                    nc.scalar.copy(out=ob[0:orows, :], in_=ps[0:orows, :])
                    nc.sync.dma_start(
                        out=o_d[g, os_ : os_ + orows, :], in_=ob[0:orows, :]
                    )
                    idx += 1

    nc.compile()
    return nc


_LOCK = threading.Lock()
_CACHED = {}


def _get_nc(reps: int = 1):
    with _LOCK:
        key = ("nc", reps)
        if key not in _CACHED:
            _CACHED[key] = _build(reps)
        return _CACHED[key]


def _postprocess(out48_f16: np.ndarray) -> np.ndarray:
    out = out48_f16.astype(np.float32).reshape(B, C, H, W)
    ch = _window_counts()
    out[..., 0:R] *= (9.0 / ch[0:R])[None, None, None, :]
    out[..., W - R : W] *= (9.0 / ch[H - R : H])[None, None, None, :]
    return out


def run(x: np.ndarray, trace: bool = False, reps: int = 1):
    from concourse.bass_utils import run_bass_kernel_spmd

    assert x.shape == (B, C, H, W), x.shape
    x48 = np.ascontiguousarray(x.reshape(IMGS, H, W), dtype=np.float32)
    wts = _consts()
    in_maps = [
        {
            "x": np.ascontiguousarray(
                x48[IMGS_PER_CORE * c : IMGS_PER_CORE * (c + 1)]
            ),
            "wts": wts,
        }
        for c in range(NCORES)
    ]
    nc = _get_nc(reps)
    res = run_bass_kernel_spmd(
        nc, in_maps, core_ids=list(range(NCORES)), trace=trace
    )
    out48 = np.concatenate([r["out"] for r in res.results], axis=0)
    return _postprocess(out48), res


def kernel(x: np.ndarray) -> np.ndarray:
    out, _ = run(x, trace=False)
    return out


# revision 8
# speedup vs baseline: 1.0398x; 1.0398x over previous
"""BoxFilter 9x9 mean, TRN2 x8 — v6: prefix-scan horizontal, 2 wide matmuls.

Per 128-row input tile: one DVE tensor_tensor_scan builds the horizontal
inclusive cumsum C (fp32 state, f16 out) over the zero-padded row; the 9-tap
horizontal window is then h9[c] = C[c+9] - C[c], folded into the PE pass as
two 1024-wide matmuls with +W and -W banded weights (vertical 9-window and
1/(9*count_v) row normalization folded into the f16 weights). One Act
cast-copy PSUM->SBUF f16, one store DMA per block. Loads ride SWDGE (Pool),
stores ride SP HWDGE, so no engine queue is oversubscribed and the serialized
DMA-engine device (~360 B/ns, destination-side bytes) stays the bottleneck.
"""

import threading

import numpy as np

NCORES = 8
B, C, H, W = 16, 3, 1024, 1024
IMGS = B * C
IMGS_PER_CORE = IMGS // NCORES
R = 4
OB = 120  # output rows per full block
NFULL = H // OB  # 8 full blocks
LASTO = H - NFULL * OB  # 64
XPW = W + 16  # padded row width: 5 left zeros, x, 11 right (>=4 read) zeros
CW = W + 9  # cumsum width read by the matmuls

# per-image block table: (out_start, out_rows, in_start, in_rows, w_idx)
BLOCKS = []
BLOCKS.append((0, OB, 0, 124, 0))
for I in range(1, NFULL):
    BLOCKS.append((OB * I, OB, OB * I - R, 128, 1))
BLOCKS.append((H - LASTO, LASTO, H - 96, 96, 2))


def _window_counts():
    r = np.arange(H)
    return (np.minimum(r + R, H - 1) - np.maximum(r - R, 0) + 1).astype(np.float32)


def _consts():
    """Banded vertical-window weights with row normalization folded in.

    Layout [128, 768]: cols [wi*128, wi*128+orows) hold +W for block type wi,
    cols [384+wi*128, ...) hold -W. W[k, m] = 1/(9*count_v[row]) on the band.
    """
    ch = _window_counts()
    wts = np.zeros((128, 768), np.float16)
    for wi, (os_, orows, is_, irows) in (
        (0, (0, OB, 0, 124)),
        (1, (OB, OB, OB - R, 128)),
        (2, (H - LASTO, LASTO, H - 96, 96)),
    ):
        k = np.arange(irows)[:, None]
        m = np.arange(orows)[None, :]
        gr = os_ + m
        lo = np.maximum(gr - R, 0) - is_
        hi = np.minimum(gr + R, H - 1) - is_
        band = ((k >= lo) & (k <= hi)).astype(np.float32)
        w = (band * (1.0 / (9.0 * ch[os_ : os_ + orows]))[None, :]).astype(
            np.float16
        )
        wts[0:irows, wi * 128 : wi * 128 + orows] = w
        wts[0:irows, 384 + wi * 128 : 384 + wi * 128 + orows] = -w
    return wts


def _build(reps: int = 1):
    import concourse.bacc as bacc
    import concourse.mybir as mybir
    import concourse.tile as tile

    f32 = mybir.dt.float32
    f16 = mybir.dt.float16
    ADD = mybir.AluOpType.add
    BYP = mybir.AluOpType.bypass

    nc = bacc.Bacc("TRN2", target_bir_lowering=False, debug=False, num_devices=NCORES)
    x_d = nc.declare_dram_parameter("x", [IMGS_PER_CORE, H, W], f32, isOutput=False)
    wts_d = nc.declare_dram_parameter("wts", [128, 768], f16, isOutput=False)
    o_d = nc.declare_dram_parameter("out", [IMGS_PER_CORE, H, W], f16, isOutput=True)

    NB = 8  # rotation depth for SBUF tiles
    NPS = 4  # PSUM tiles (2 banks each)

    with tile.TileContext(nc) as tc, (
        tc.tile_pool(name="consts", bufs=1)
    ) as cpool, tc.tile_pool(name="bufs", bufs=1) as bpool, (
        tc.tile_pool(name="psum", bufs=1, space="PSUM")
    ) as ppool:
        w_sb = cpool.tile([128, 768], f16, name="w_sb")
        nc.sync.dma_start(out=w_sb[:], in_=wts_d[:])

        xps = [bpool.tile([128, XPW], f16, name=f"xp{i}") for i in range(NB)]
        cts = [bpool.tile([128, 1036], f16, name=f"ct{i}") for i in range(NB)]
        obs = [bpool.tile([128, W], f16, name=f"ob{i}") for i in range(NB)]
        pss = [ppool.tile([128, W], f32, name=f"psm{i}") for i in range(NPS)]

        # pad columns are written once; loads only touch cols [5, W+5)
        for i in range(NB):
            nc.vector.memset(xps[i][0:128, 0:5], 0.0)
            nc.vector.memset(xps[i][0:128, W + 5 : XPW], 0.0)

        idx = 0
        for _ in range(reps):
            for g in range(IMGS_PER_CORE):
                for os_, orows, is_, irows, wi in BLOCKS:
                    xp = xps[idx % NB]
                    ct = cts[idx % NB]
                    ob = obs[idx % NB]
                    ps = pss[idx % NPS]
                    nc.gpsimd.dma_start(
                        out=xp[0:irows, 5 : W + 5], in_=x_d[g, is_ : is_ + irows, :]
                    )
                    nc.vector.tensor_tensor_scan(
                        ct[0:irows, 0:CW],
                        xp[0:irows, 0:CW],
                        xp[0:irows, 0:CW],
                        0.0,
                        ADD,
                        BYP,
                    )
                    for j0 in (0, 512):
                        nc.tensor.matmul(
                            ps[0:orows, j0 : j0 + 512],
                            w_sb[0:irows, wi * 128 : wi * 128 + orows],
                            ct[0:irows, j0 + 9 : j0 + 521],
                            start=True,
                            stop=False,
                        )
                        nc.tensor.matmul(
                            ps[0:orows, j0 : j0 + 512],
                            w_sb[0:irows, 384 + wi * 128 : 384 + wi * 128 + orows],
                            ct[0:irows, j0 : j0 + 512],
                            start=False,
                            stop=True,
                        )
                    nc.scalar.copy(out=ob[0:orows, :], in_=ps[0:orows, :])
                    nc.sync.dma_start(
                        out=o_d[g, os_ : os_ + orows, :], in_=ob[0:orows, :]
                    )
                    idx += 1

    nc.compile()
    return nc


_LOCK = threading.Lock()
_CACHED = {}


def _get_nc(reps: int = 1):
    with _LOCK:
        key = ("nc", reps)
        if key not in _CACHED:
            _CACHED[key] = _build(reps)
        return _CACHED[key]


def _postprocess(out48_f16: np.ndarray) -> np.ndarray:
    out = out48_f16.astype(np.float32).reshape(B, C, H, W)
    ch = _window_counts()
    out[..., 0:R] *= (9.0 / ch[0:R])[None, None, None, :]
    out[..., W - R : W] *= (9.0 / ch[H - R : H])[None, None, None, :]
    return out


def run(x: np.ndarray, trace: bool = False, reps: int = 1):
    from concourse.bass_utils import run_bass_kernel_spmd

    assert x.shape == (B, C, H, W), x.shape
    x48 = np.ascontiguousarray(x.reshape(IMGS, H, W), dtype=np.float32)
    wts = _consts()
    in_maps = [
        {
            "x": np.ascontiguousarray(
                x48[IMGS_PER_CORE * c : IMGS_PER_CORE * (c + 1)]
            ),
            "wts": wts,
        }
        for c in range(NCORES)
    ]
    nc = _get_nc(reps)
    res = run_bass_kernel_spmd(
        nc, in_maps, core_ids=list(range(NCORES)), trace=trace
    )
    out48 = np.concatenate([r["out"] for r in res.results], axis=0)
    return _postprocess(out48), res


def kernel(x: np.ndarray) -> np.ndarray:
    out, _ = run(x, trace=False)
    return out
